# revision 1
# baseline (speedup 1.0000x reference)
"""CrossAttention TRN2 kernel.

Problem (hardcoded shapes):
  x    [4, 2048, 1024], cond [4, 2048, 1024]
  Wq/Wk/Wv [1024, 1024], Wo [1024, 1024], bo [1024]
  out = softmax((x@Wq) reshaped to 8 heads of 128 @ (cond@Wk)^T * 0.125) @ (cond@Wv) @ Wo + bo

Sharding: 8 cores = (batch b in 0..3) x (query-half ih in 0..1).
Each core computes 1024 query rows for one batch, all 8 heads; K/V projection
for that batch is replicated across the 2 cores sharing it. No collectives.

Device layouts (host pre-transposes):
  xT    [1024 cdim, 1024 i]   = x[b, ih*1024:(ih+1)*1024, :].T
  condT [1024 cdim, 2048 j]   = cond[b].T
  Weights as-is (Wq pre-scaled by 0.125). All matmuls run as float32r
  (full PE rate at free dim >= 256, ~1e-4 relative accuracy).

Per-head flash attention with scores kept transposed [j, i] so softmax
denominator comes from a ones-stationary matmul and attn@v needs no transpose.
"""
import numpy as np

import concourse.bass as bass
import concourse.bacc as bacc
import concourse.tile as tile
from concourse import bass_isa, mybir
from concourse.bass_utils import run_bass_kernel_spmd

F32 = mybir.dt.float32
F32R = mybir.dt.float32r
EXP = mybir.ActivationFunctionType.Exp

B, NQ, NK, D = 4, 2048, 2048, 1024   # D = query_dim = cond_dim = inner_dim = out_dim
H, DH = 8, 128                        # heads, per-head dim
SCALE = 64 ** -0.5                    # reference uses dim_head=64 for the scale
NCORES = 8
IQ = NQ // 2                          # query rows per core (1024)
KT = D // 128                         # contraction tiles (8)
GROUPS, HPG = 4, 2                    # head groups of 2 heads
JT = NK // 128                        # key tiles (16)


def build_nc():
    nc = bacc.Bacc()
    xT = nc.declare_dram_parameter("xT", [D, IQ], F32R, isOutput=False)
    condT = nc.declare_dram_parameter("condT", [D, NK], F32R, isOutput=False)
    wq = nc.declare_dram_parameter("wq", [D, D], F32R, isOutput=False)
    wk = nc.declare_dram_parameter("wk", [D, D], F32R, isOutput=False)
    wv = nc.declare_dram_parameter("wv", [D, D], F32R, isOutput=False)
    wo = nc.declare_dram_parameter("wo", [D, D], F32R, isOutput=False)
    bo = nc.declare_dram_parameter("bo", [1, D], F32, isOutput=False)
    out = nc.declare_dram_parameter("out", [IQ, D], F32, isOutput=True)

    with tile.TileContext(nc) as tc:
        with (
            nc.allow_low_precision(reason="float32r matmul operands are intended"),
            tc.tile_pool(name="const", bufs=1) as const,
            tc.tile_pool(name="big", bufs=1) as big,
            tc.tile_pool(name="grp", bufs=1) as grp,
            tc.tile_pool(name="xstream", bufs=2) as xstream,
            tc.tile_pool(name="expp", bufs=3) as expp,
            tc.tile_pool(name="small", bufs=1) as small,
            tc.tile_pool(name="ostage", bufs=2) as ostage,
            tc.tile_pool(name="ps", bufs=1, space="PSUM") as ps,
        ):
            bo_bc = const.tile([128, D], F32)
            nc.sync.dma_start(out=bo_bc, in_=bo[:, :].to_broadcast((128, D)))

            # resident condT: 8 tiles [128, 2048] (loaded after group-0 weights
            # so the first projection matmuls are not stuck behind 8MB of DMA)
            ct = big.tile([128, KT, NK], F32R, tag="ct")

            # attention output, transposed layout: [dh, head, i]
            attT = big.tile([128, H, IQ], F32R)

            for g in range(GROUPS):
                c0 = g * HPG * DH          # first inner column of this group
                gw = HPG * DH              # 256

                # group weights resident: [128, KT, 256]
                wq_g = grp.tile([128, KT, gw], F32R, tag="wq_g")
                wk_g = grp.tile([128, KT, gw], F32R, tag="wk_g")
                wv_g = grp.tile([128, KT, gw], F32R, tag="wv_g")
                for k in range(KT):
                    rows = slice(k * 128, (k + 1) * 128)
                    nc.sync.dma_start(out=wq_g[:, k, :], in_=wq[rows, c0:c0 + gw])
                    nc.sync.dma_start(out=wk_g[:, k, :], in_=wk[rows, c0:c0 + gw])
                    nc.sync.dma_start(out=wv_g[:, k, :], in_=wv[rows, c0:c0 + gw])
                if g == 0:
                    for k in range(KT):
                        for jh2 in range(2):
                            nc.sync.dma_start(
                                out=ct[:, k, jh2 * 1024:(jh2 + 1) * 1024],
                                in_=condT[k * 128:(k + 1) * 128,
                                          jh2 * 1024:(jh2 + 1) * 1024])

                qT_g = grp.tile([128, HPG, IQ], F32R, tag="qT_g")
                kT_g = grp.tile([128, HPG, NK], F32R, tag="kT_g")
                v_g = grp.tile([128, JT, gw], F32R, tag="v_g")

                # ---- q projection: qT_g[:, mh, ih*512:] = Wq_slice.T @ xT ----
                for ih in range(IQ // 512):
                    accs = [ps.tile([128, 512], F32, tag="pp", bufs=2,
                                    name=f"accq_{ih}_{m}") for m in range(HPG)]
                    for k in range(KT):
                        xk = xstream.tile([128, 512], F32R, tag="xk",
                                          name=f"xk_{ih}_{k}")
                        nc.sync.dma_start(
                            out=xk,
                            in_=xT[k * 128:(k + 1) * 128, ih * 512:(ih + 1) * 512])
                        for mh in range(HPG):
                            nc.tensor.matmul(
                                accs[mh],
                                (wq_g[:, k, mh * DH:(mh + 1) * DH]),
                                (xk),
                                start=(k == 0), stop=(k == KT - 1))
                    for mh in range(HPG):
                        nc.vector.tensor_copy(qT_g[:, mh, ih * 512:(ih + 1) * 512], accs[mh])

                # ---- k projection: kT_g[:, mh, jh*512:] = Wk_slice.T @ condT ----
                for jh in range(NK // 512):
                    accs = [ps.tile([128, 512], F32, tag="pp", bufs=2,
                                    name=f"acck_{jh}_{m}") for m in range(HPG)]
                    for k in range(KT):
                        for mh in range(HPG):
                            nc.tensor.matmul(
                                accs[mh],
                                (wk_g[:, k, mh * DH:(mh + 1) * DH]),
                                (ct[:, k, jh * 512:(jh + 1) * 512]),
                                start=(k == 0), stop=(k == KT - 1))
                    for mh in range(HPG):
                        nc.vector.tensor_copy(kT_g[:, mh, jh * 512:(jh + 1) * 512], accs[mh])

                # ---- v projection: v_g[:, jt, :] = condT_jt.T @ Wv_slice ----
                for jt in range(JT):
                    acc = ps.tile([128, gw], F32, tag="pp", bufs=2)
                    for k in range(KT):
                        nc.tensor.matmul(
                            acc,
                            (ct[:, k, jt * 128:(jt + 1) * 128]),
                            (wv_g[:, k, :]),
                            start=(k == 0), stop=(k == KT - 1))
                    nc.vector.tensor_copy(v_g[:, jt, :], acc)

                # ---- attention per head ----
                for hg in range(HPG):
                    h = g * HPG + hg
                    avs = [ps.tile([128, 512], F32, tag="av", bufs=2,
                                   name=f"av_{h}_{i}") for i in range(IQ // 512)]
                    den_s = small.tile([128, IQ], F32, tag="den_s",
                                       name=f"den_s_{h}")
                    for jt in range(JT):
                        sc = ps.tile([128, IQ], F32, tag="sc", bufs=2)
                        for ih in range(IQ // 512):
                            nc.tensor.matmul(
                                sc[:, ih * 512:(ih + 1) * 512],
                                kT_g[:, hg, jt * 128:(jt + 1) * 128],
                                qT_g[:, hg, ih * 512:(ih + 1) * 512],
                                start=True, stop=True)
                        esc = expp.tile([128, IQ], F32R, tag="esc")
                        nc.scalar.activation(esc, sc, EXP)
                        for ih in range(IQ // 512):
                            nc.tensor.matmul(
                                avs[ih],
                                v_g[:, jt, hg * DH:(hg + 1) * DH],
                                esc[:, ih * 512:(ih + 1) * 512],
                                start=(jt == 0), stop=(jt == JT - 1))
                        with tc.high_priority():
                            if jt == 0:
                                nc.vector.tensor_copy(den_s, esc.bitcast(F32))
                            else:
                                nc.vector.tensor_add(den_s, den_s, esc.bitcast(F32))
                    den_bc = small.tile([128, IQ], F32, tag="den_bc",
                                        name=f"den_bc_{h}")
                    with tc.high_priority():
                        nc.gpsimd.partition_all_reduce(
                            den_bc, den_s, 128, bass_isa.ReduceOp.add)
                        nc.vector.reciprocal(den_bc, den_bc)
                        for ih in range(IQ // 512):
                            nc.vector.tensor_mul(
                                attT[:, h, ih * 512:(ih + 1) * 512], avs[ih],
                                den_bc[:, ih * 512:(ih + 1) * 512])

            # ---- output projection: out[it*128:, nh*512:] = attT.T @ Wo + bo ----
            wo_r = big.tile([128, KT, D], F32R, tag="ct")
            for k in range(KT):
                nc.sync.dma_start(out=wo_r[:, k, :], in_=wo[k * 128:(k + 1) * 128, :])
            for it in range(IQ // 128):
                for nh in range(D // 512):
                    fo = ps.tile([128, 512], F32, tag="pp", bufs=2)
                    for k in range(KT):
                        nc.tensor.matmul(
                            fo,
                            (attT[:, k, it * 128:(it + 1) * 128]),
                            (wo_r[:, k, nh * 512:(nh + 1) * 512]),
                            start=(k == 0), stop=(k == KT - 1))
                    fo_sb = ostage.tile([128, 512], F32, tag="fo_sb")
                    nc.vector.tensor_add(fo_sb, fo, bo_bc[:, nh * 512:(nh + 1) * 512])
                    nc.sync.dma_start(
                        out=out[it * 128:(it + 1) * 128, nh * 512:(nh + 1) * 512],
                        in_=fo_sb)
    nc.finalize()
    return nc


_NC_CACHE = None


def _get_nc():
    global _NC_CACHE
    if _NC_CACHE is None:
        _NC_CACHE = build_nc()
    return _NC_CACHE


def make_in_maps(x, cond, Wq, Wk, Wv, Wo, bo):
    wq_s = np.ascontiguousarray(Wq * SCALE, dtype=np.float32)
    wk_c = np.ascontiguousarray(Wk, dtype=np.float32)
    wv_c = np.ascontiguousarray(Wv, dtype=np.float32)
    wo_c = np.ascontiguousarray(Wo, dtype=np.float32)
    bo_c = np.ascontiguousarray(bo, dtype=np.float32).reshape(1, D)
    in_maps = []
    for c in range(NCORES):
        b, ih = c // 2, c % 2
        in_maps.append({
            "xT": np.ascontiguousarray(x[b, ih * IQ:(ih + 1) * IQ, :].T),
            "condT": np.ascontiguousarray(cond[b].T),
            "wq": wq_s, "wk": wk_c, "wv": wv_c, "wo": wo_c, "bo": bo_c,
        })
    return in_maps


def kernel(x, cond, Wq, Wk, Wv, Wo, bo, _trace=False, _trace_kwargs=None):
    x = np.asarray(x, dtype=np.float32)
    cond = np.asarray(cond, dtype=np.float32)
    nc = _get_nc()
    in_maps = make_in_maps(x, cond,
                           np.asarray(Wq, np.float32), np.asarray(Wk, np.float32),
                           np.asarray(Wv, np.float32), np.asarray(Wo, np.float32),
                           np.asarray(bo, np.float32))
    kw = {}
    if _trace:
        kw = {"trace": True, "trace_kwargs": _trace_kwargs or {}}
    res = run_bass_kernel_spmd(nc, in_maps, list(range(NCORES)), **kw)
    out = np.empty((B, NQ, D), dtype=np.float32)
    for c in range(NCORES):
        b, ih = c // 2, c % 2
        out[b, ih * IQ:(ih + 1) * IQ, :] = res.results[c]["out"]
    if _trace:
        return out, res
    return out


if __name__ == "__main__":
    # quick numeric self-check against numpy (no jax needed)
    rng = np.random.default_rng(0)
    s = 0.02
    x = rng.standard_normal((B, NQ, D), dtype=np.float32)
    cond = rng.standard_normal((B, NK, D), dtype=np.float32)
    Wq = (rng.standard_normal((D, D), dtype=np.float32) * s)
    Wk = (rng.standard_normal((D, D), dtype=np.float32) * s)
    Wv = (rng.standard_normal((D, D), dtype=np.float32) * s)
    Wo = (rng.standard_normal((D, D), dtype=np.float32) * s)
    bo = (rng.standard_normal((D,), dtype=np.float32) * s)

    def ref_np(x, cond):
        q = (x @ Wq).reshape(B, NQ, H, DH).transpose(0, 2, 1, 3)
        k = (cond @ Wk).reshape(B, NK, H, DH).transpose(0, 2, 1, 3)
        v = (cond @ Wv).reshape(B, NK, H, DH).transpose(0, 2, 1, 3)
        sim = np.einsum('bhid,bhjd->bhij', q, k) * SCALE
        sim = sim - sim.max(axis=-1, keepdims=True)
        a = np.exp(sim)
        a = a / a.sum(axis=-1, keepdims=True)
        o = np.einsum('bhij,bhjd->bhid', a, v)
        o = o.transpose(0, 2, 1, 3).reshape(B, NQ, D)
        return o @ Wo + bo

    import time
    t0 = time.time()
    got = kernel(x=x, cond=cond, Wq=Wq, Wk=Wk, Wv=Wv, Wo=Wo, bo=bo)
    print(f"kernel run {time.time()-t0:.1f}s")
    exp = ref_np(x.astype(np.float64), cond.astype(np.float64))
    err = np.abs(got - exp)
    rel = np.linalg.norm(got - exp) / np.linalg.norm(exp)
    print(f"rel_l2={rel:.3e} absmax_rel={err.max()/np.abs(exp).max():.3e}")



# revision 2
# speedup vs baseline: 1.3922x; 1.3922x over previous
"""CrossAttention TRN2 kernel (fp16 operands, fp32 accumulation).

Problem (hardcoded shapes):
  x    [4, 2048, 1024], cond [4, 2048, 1024]
  Wq/Wk/Wv [1024, 1024], Wo [1024, 1024], bo [1024]
  out = softmax((x@Wq) 8 heads of 128 @ (cond@Wk)^T * 0.125) @ (cond@Wv) @ Wo + bo

Sharding: 8 cores = (batch b in 0..3) x (query-half ih in 0..1).
Each core computes 1024 query rows for one batch, all 8 heads; K/V projection
for that batch is replicated across the 2 cores sharing it. No collectives.

Layout/schedule notes:
  - All matmul operands are fp16 in SBUF (1 cy/row PE rate, half the DMA and
    LDWEIGHTS cost of fp32, no fp32-HIGH power throttling); accumulation stays
    fp32 in PSUM.
  - xT and condT are resident in SBUF; weights are streamed per head-group
    (two groups in flight) from host-packed contiguous blocks.
  - All four groups' Q projections are hoisted to the start so the tensor
    engine has work while condT (4MB) streams in.
  - Scores are kept transposed [j, i]; softmax denominator partials are
    accumulated on DVE in fp16 (4x perf mode), partition-reduced with a
    ones-stationary matmul, inverted with reciprocal_approx_fast.
  - PSUM budget: pp(2x[128,512]) + av(2x[128,512]) + sc(2x[128,1024]) = 8 banks.
"""
import numpy as np

import concourse.bass as bass
import concourse.bacc as bacc
import concourse.tile as tile
from concourse import bass_isa, mybir
from concourse.bass_utils import run_bass_kernel_spmd

F32 = mybir.dt.float32
F16 = mybir.dt.float16
EXP = mybir.ActivationFunctionType.Exp

B, NQ, NK, D = 4, 2048, 2048, 1024   # D = query_dim = cond_dim = inner_dim = out_dim
H, DH = 8, 128                        # heads, per-head dim
SCALE = 64 ** -0.5                    # reference uses dim_head=64 for the scale
NCORES = 8
IQ = NQ // 2                          # query rows per core (1024)
KT = D // 128                         # contraction tiles (8)
GROUPS, HPG = 4, 2                    # head groups of 2 heads
JT = NK // 128                        # key tiles (16)
GW = HPG * DH                         # 256 inner columns per group


def build_nc():
    nc = bacc.Bacc()
    xT = nc.declare_dram_parameter("xT", [KT, 128, IQ], F16, isOutput=False)
    condT = nc.declare_dram_parameter("condT", [KT, 128, NK], F16, isOutput=False)
    wq = nc.declare_dram_parameter("wq", [KT, GROUPS, 128, GW], F16, isOutput=False)
    wk = nc.declare_dram_parameter("wk", [KT, GROUPS, 128, GW], F16, isOutput=False)
    wv = nc.declare_dram_parameter("wv", [KT, GROUPS, 128, GW], F16, isOutput=False)
    wo = nc.declare_dram_parameter("wo", [KT, 128, D], F16, isOutput=False)
    bo = nc.declare_dram_parameter("bo", [1, D], F32, isOutput=False)
    out = nc.declare_dram_parameter("out", [IQ, D], F32, isOutput=True)

    with tile.TileContext(nc) as tc:
        with (
            nc.allow_low_precision(reason="fp16 matmul operands are intended"),
            tc.tile_pool(name="const", bufs=1) as const,
            tc.tile_pool(name="big", bufs=1) as big,
            tc.tile_pool(name="grp", bufs=2) as grp,
            tc.tile_pool(name="expp", bufs=4) as expp,
            tc.tile_pool(name="den", bufs=2) as denp,
            tc.tile_pool(name="ostage", bufs=2) as ostage,
            tc.tile_pool(name="ps", bufs=1, space="PSUM") as ps,
        ):
            bo_bc = const.tile([128, D], F32)
            nc.sync.dma_start(out=bo_bc, in_=bo[:, :].to_broadcast((128, D)))
            ones = const.tile([128, 128], F16)
            nc.vector.memset(ones, 1.0)

            # resident inputs / q / attention output
            wq_all = big.tile([128, KT, D], F16, tag="wq_all")
            xt = big.tile([128, KT, IQ], F16, tag="xt")
            ct = big.tile([128, KT, NK], F16, tag="ct")
            qT = big.tile([128, H, IQ], F16, tag="qT")
            attT = big.tile([128, H, IQ], F16, tag="attT")

            # startup DMA order: wq+xt (Q proj feeds first), then ct, group-0
            # K/V weights issued below in the group loop body.
            for k in range(KT):
                for g in range(GROUPS):
                    nc.sync.dma_start(out=wq_all[:, k, g * GW:(g + 1) * GW],
                                      in_=wq[k, g, :, :])
                nc.sync.dma_start(out=xt[:, k, :], in_=xT[k, :, :])
            for k in range(KT):
                nc.sync.dma_start(out=ct[:, k, :], in_=condT[k, :, :])

            # ---- all Q projections up front: qT[:, h, :] = Wq_h.T @ xT ----
            for g in range(GROUPS):
                for ih in range(IQ // 512):
                    accs = [ps.tile([128, 512], F32, tag="pp", bufs=2,
                                    name=f"accq_{g}_{ih}_{m}") for m in range(HPG)]
                    for k in range(KT):
                        for mh in range(HPG):
                            nc.tensor.matmul(
                                accs[mh],
                                wq_all[:, k, g * GW + mh * DH:g * GW + (mh + 1) * DH],
                                xt[:, k, ih * 512:(ih + 1) * 512],
                                start=(k == 0), stop=(k == KT - 1))
                    for mh in range(HPG):
                        nc.vector.tensor_copy(
                            qT[:, g * HPG + mh, ih * 512:(ih + 1) * 512], accs[mh])

            for g in range(GROUPS):
                wk_g = grp.tile([128, KT, GW], F16, tag="wk_g")
                wv_g = grp.tile([128, KT, GW], F16, tag="wv_g")
                for k in range(KT):
                    nc.sync.dma_start(out=wk_g[:, k, :], in_=wk[k, g, :, :])
                for k in range(KT):
                    nc.sync.dma_start(out=wv_g[:, k, :], in_=wv[k, g, :, :])

                kT_g = grp.tile([128, HPG, NK], F16, tag="kT_g")
                v_g = grp.tile([128, JT, GW], F16, tag="v_g")

                # ---- k projection: kT_g[:, mh, jh*512:] = Wk_slice.T @ condT ----
                for jh in range(NK // 512):
                    accs = [ps.tile([128, 512], F32, tag="pp", bufs=2,
                                    name=f"acck_{g}_{jh}_{m}") for m in range(HPG)]
                    for k in range(KT):
                        for mh in range(HPG):
                            nc.tensor.matmul(
                                accs[mh],
                                wk_g[:, k, mh * DH:(mh + 1) * DH],
                                ct[:, k, jh * 512:(jh + 1) * 512],
                                start=(k == 0), stop=(k == KT - 1))
                    for mh in range(HPG):
                        nc.vector.tensor_copy(kT_g[:, mh, jh * 512:(jh + 1) * 512], accs[mh])

                # ---- v projection: v_g[:, jt, :] = condT_jt.T @ Wv_slice ----
                for jt in range(JT):
                    acc = ps.tile([128, GW], F32, tag="pp", bufs=2,
                                  name=f"accv_{g}_{jt}")
                    for k in range(KT):
                        nc.tensor.matmul(
                            acc,
                            ct[:, k, jt * 128:(jt + 1) * 128],
                            wv_g[:, k, :],
                            start=(k == 0), stop=(k == KT - 1))
                    # scalar engine does the v copies (vector handles den adds)
                    nc.scalar.copy(v_g[:, jt, :], acc)

                # ---- attention per head, scores transposed [j, i] ----
                for hg in range(HPG):
                    h = g * HPG + hg
                    avs = [ps.tile([128, 512], F32, tag="av", bufs=2,
                                   name=f"av_{h}_{i}") for i in range(IQ // 512)]
                    den_s = denp.tile([128, IQ], F16, tag="den_s",
                                      name=f"den_s_{h}")
                    for jt in range(JT):
                        sc = ps.tile([128, IQ], F32, tag="sc", bufs=2,
                                     name=f"sc_{h}_{jt}")
                        for ih in range(IQ // 512):
                            nc.tensor.matmul(
                                sc[:, ih * 512:(ih + 1) * 512],
                                kT_g[:, hg, jt * 128:(jt + 1) * 128],
                                qT[:, h, ih * 512:(ih + 1) * 512],
                                start=True, stop=True)
                        esc = expp.tile([128, IQ], F16, tag="esc",
                                        name=f"esc_{h}_{jt}")
                        nc.scalar.activation(esc, sc, EXP)
                        for ih in range(IQ // 512):
                            nc.tensor.matmul(
                                avs[ih],
                                v_g[:, jt, hg * DH:(hg + 1) * DH],
                                esc[:, ih * 512:(ih + 1) * 512],
                                start=(jt == 0), stop=(jt == JT - 1))
                        with tc.high_priority():
                            if jt == 0:
                                nc.vector.tensor_copy(den_s, esc)
                            else:
                                nc.vector.tensor_add(den_s, den_s, esc)
                    # partition-reduce den via ones-matmul, then fast reciprocal
                    den_bc = ps.tile([128, IQ], F32, tag="sc", bufs=2,
                                     name=f"den_bc_{h}")
                    den_rec = denp.tile([128, IQ], F32, tag="den_rec",
                                        name=f"den_rec_{h}")
                    with tc.high_priority():
                        for ih in range(IQ // 512):
                            nc.tensor.matmul(
                                den_bc[:, ih * 512:(ih + 1) * 512],
                                ones,
                                den_s[:, ih * 512:(ih + 1) * 512],
                                start=True, stop=True)
                        nc.vector.reciprocal_approx_fast(out=den_rec, in_=den_bc)
                        for ih in range(IQ // 512):
                            nc.vector.tensor_mul(
                                attT[:, h, ih * 512:(ih + 1) * 512], avs[ih],
                                den_rec[:, ih * 512:(ih + 1) * 512])

            # ---- output projection: out[it*128:, nh*512:] = attT.T @ Wo + bo ----
            wo_s = big.tile([128, KT, D], F16, tag="wq_all")
            for k in range(KT):
                nc.sync.dma_start(out=wo_s[:, k, :], in_=wo[k, :, :])
            for it in range(IQ // 128):
                for nh in range(D // 512):
                    fo = ps.tile([128, 512], F32, tag="pp", bufs=2,
                                 name=f"fo_{it}_{nh}")
                    for k in range(KT):
                        nc.tensor.matmul(
                            fo,
                            attT[:, k, it * 128:(it + 1) * 128],
                            wo_s[:, k, nh * 512:(nh + 1) * 512],
                            start=(k == 0), stop=(k == KT - 1))
                    fo_sb = ostage.tile([128, 512], F32, tag="fo_sb")
                    nc.vector.tensor_add(fo_sb, fo, bo_bc[:, nh * 512:(nh + 1) * 512])
                    nc.sync.dma_start(
                        out=out[it * 128:(it + 1) * 128, nh * 512:(nh + 1) * 512],
                        in_=fo_sb)
    nc.finalize()
    return nc


_NC_CACHE = None


def _get_nc():
    global _NC_CACHE
    if _NC_CACHE is None:
        _NC_CACHE = build_nc()
    return _NC_CACHE


def make_in_maps(x, cond, Wq, Wk, Wv, Wo, bo):
    def pack_w(w, scale=1.0):
        w = (np.asarray(w, np.float32) * scale).astype(np.float16)
        # [1024, 1024] -> [KT, 128, GROUPS, GW] -> [KT, GROUPS, 128, GW]
        return np.ascontiguousarray(
            w.reshape(KT, 128, GROUPS, GW).transpose(0, 2, 1, 3))

    wq_p = pack_w(Wq, SCALE)
    wk_p = pack_w(Wk)
    wv_p = pack_w(Wv)
    wo_p = np.ascontiguousarray(
        np.asarray(Wo, np.float32).astype(np.float16).reshape(KT, 128, D))
    bo_c = np.ascontiguousarray(bo, dtype=np.float32).reshape(1, D)
    x16 = np.asarray(x, np.float32).astype(np.float16)
    c16 = np.asarray(cond, np.float32).astype(np.float16)
    in_maps = []
    for c in range(NCORES):
        b, ih = c // 2, c % 2
        in_maps.append({
            "xT": np.ascontiguousarray(
                x16[b, ih * IQ:(ih + 1) * IQ, :].T).reshape(KT, 128, IQ),
            "condT": np.ascontiguousarray(c16[b].T).reshape(KT, 128, NK),
            "wq": wq_p, "wk": wk_p, "wv": wv_p, "wo": wo_p, "bo": bo_c,
        })
    return in_maps


def kernel(x, cond, Wq, Wk, Wv, Wo, bo, _trace=False, _trace_kwargs=None):
    nc = _get_nc()
    in_maps = make_in_maps(x, cond, Wq, Wk, Wv, Wo, bo)
    kw = {}
    if _trace:
        kw = {"trace": True, "trace_kwargs": _trace_kwargs or {}}
    res = run_bass_kernel_spmd(nc, in_maps, list(range(NCORES)), **kw)
    out = np.empty((B, NQ, D), dtype=np.float32)
    for c in range(NCORES):
        b, ih = c // 2, c % 2
        out[b, ih * IQ:(ih + 1) * IQ, :] = res.results[c]["out"]
    if _trace:
        return out, res
    return out


if __name__ == "__main__":
    # quick numeric self-check against numpy (no jax needed)
    rng = np.random.default_rng(0)
    s = 0.02
    x = rng.standard_normal((B, NQ, D), dtype=np.float32)
    cond = rng.standard_normal((B, NK, D), dtype=np.float32)
    Wq = (rng.standard_normal((D, D), dtype=np.float32) * s)
    Wk = (rng.standard_normal((D, D), dtype=np.float32) * s)
    Wv = (rng.standard_normal((D, D), dtype=np.float32) * s)
    Wo = (rng.standard_normal((D, D), dtype=np.float32) * s)
    bo = (rng.standard_normal((D,), dtype=np.float32) * s)

    def ref_np(x, cond):
        q = (x @ Wq).reshape(B, NQ, H, DH).transpose(0, 2, 1, 3)
        k = (cond @ Wk).reshape(B, NK, H, DH).transpose(0, 2, 1, 3)
        v = (cond @ Wv).reshape(B, NK, H, DH).transpose(0, 2, 1, 3)
        sim = np.einsum('bhid,bhjd->bhij', q, k) * SCALE
        sim = sim - sim.max(axis=-1, keepdims=True)
        a = np.exp(sim)
        a = a / a.sum(axis=-1, keepdims=True)
        o = np.einsum('bhij,bhjd->bhid', a, v)
        o = o.transpose(0, 2, 1, 3).reshape(B, NQ, D)
        return o @ Wo + bo

    import time
    t0 = time.time()
    got = kernel(x=x, cond=cond, Wq=Wq, Wk=Wk, Wv=Wv, Wo=Wo, bo=bo)
    print(f"kernel run {time.time()-t0:.1f}s")
    exp = ref_np(x.astype(np.float64), cond.astype(np.float64))
    err = np.abs(got - exp)
    rel = np.linalg.norm(got - exp) / np.linalg.norm(exp)
    print(f"rel_l2={rel:.3e} absmax_rel={err.max()/np.abs(exp).max():.3e}")


# revision 4
# speedup vs baseline: 1.8969x; 1.3625x over previous
"""CrossAttention TRN2 kernel (head-parallel, fp16 operands, host-summed partials).

Problem (hardcoded shapes):
  x    [4, 2048, 1024], cond [4, 2048, 1024]
  Wq/Wk/Wv [1024, 1024], Wo [1024, 1024], bo [1024]
  out = softmax((x@Wq) 8 heads of 128 @ (cond@Wk)^T * 0.125) @ (cond@Wv) @ Wo + bo

Sharding: 8 cores = (batch b in 0..3) x (head-half hh in 0..1).
Each core computes Q/K/V projections and attention for ITS 4 heads over the
full 2048 queries, then a PARTIAL output projection (contraction over its 512
inner columns only), written as fp16 [2048, 1024]. The host sums the two
partials per batch and adds the bias — no duplicated projection work and no
on-chip collectives. Total matmul rows/core: 532k (vs 655k for query-split).

Schedule notes:
  - fp16 matmul operands (1 cy/row), fp32 PSUM; PE runs ~2.0 GHz sustained.
  - xT/condT/weights fully resident; xt's SBUF is recycled for the output
    accumulator after Q projection (tag alias).
  - Scores kept transposed [j, i]; softmax denominator accumulated on DVE in
    fp16, partition-reduced via ones-matmul, inverted with
    reciprocal_approx_fast.
  - Output projection is accumulated head-by-head (single-matmul chains into
    an fp16 SBUF accumulator) so it fills the exp-bound attention phase
    instead of forming a serial tail.
  - PSUM static budget: sc 2x[128,1024] + pp 2x[128,512] + av 2x[128,512]
    = exactly 8 banks.
"""
import numpy as np

import concourse.bass as bass
import concourse.bacc as bacc
import concourse.tile as tile
from concourse import bass_isa, mybir
from concourse.bass_utils import run_bass_kernel_spmd

F32 = mybir.dt.float32
F16 = mybir.dt.float16
EXP = mybir.ActivationFunctionType.Exp

B, NQ, NK, D = 4, 2048, 2048, 1024
H, DH = 8, 128
SCALE = 64 ** -0.5
NCORES = 8
KT = D // 128                         # contraction tiles (8)
HL = 4                                # heads per core
JT = NK // 128                        # key tiles (16)
IT = NQ // 128                        # output row tiles (16)
MW = HL * DH                          # 512 inner columns per core


def build_nc():
    nc = bacc.Bacc()
    xT = nc.declare_dram_parameter("xT", [KT, 128, NQ], F16, isOutput=False)
    condT = nc.declare_dram_parameter("condT", [KT, 128, NK], F16, isOutput=False)
    wq = nc.declare_dram_parameter("wq", [KT, 128, MW], F16, isOutput=False)
    wk = nc.declare_dram_parameter("wk", [KT, 128, MW], F16, isOutput=False)
    wv = nc.declare_dram_parameter("wv", [KT, 128, MW], F16, isOutput=False)
    wo = nc.declare_dram_parameter("wo", [HL, 128, D], F16, isOutput=False)
    out = nc.declare_dram_parameter("out", [NQ, D], F16, isOutput=True)

    with tile.TileContext(nc) as tc:
        with (
            nc.allow_low_precision(reason="fp16 matmul operands are intended"),
            tc.tile_pool(name="const", bufs=1) as const,
            tc.tile_pool(name="big", bufs=1) as big,
            tc.tile_pool(name="expp", bufs=4) as expp,
            tc.tile_pool(name="den", bufs=2) as denp,
            tc.tile_pool(name="ps", bufs=1, space="PSUM") as ps,
        ):
            ones = const.tile([128, 128], F16)
            nc.vector.memset(ones, 1.0)

            wq_s = big.tile([128, KT, MW], F16, tag="wq_s")
            xt = big.tile([128, KT, NQ], F16, tag="xt")
            wk_s = big.tile([128, KT, MW], F16, tag="wk_s")
            ct = big.tile([128, KT, NK], F16, tag="ct")
            wv_s = big.tile([128, KT, MW], F16, tag="wv_s")
            wo_s = big.tile([128, HL, D], F16, tag="wo_s")
            qT = big.tile([128, HL, NQ], F16, tag="qT")
            kT = big.tile([128, HL, NK], F16, tag="kT")
            v = big.tile([128, JT, MW], F16, tag="v")
            attT = big.tile([128, HL, NQ], F16, tag="attT")

            # DMA issue order = arrival priority: Q-projection feed first.
            for k in range(KT):
                nc.sync.dma_start(out=wq_s[:, k, :], in_=wq[k, :, :])
                nc.sync.dma_start(out=xt[:, k, :], in_=xT[k, :, :])
            for k in range(KT):
                nc.sync.dma_start(out=wk_s[:, k, :], in_=wk[k, :, :])
                nc.sync.dma_start(out=ct[:, k, :], in_=condT[k, :, :])
            for k in range(KT):
                nc.sync.dma_start(out=wv_s[:, k, :], in_=wv[k, :, :])
            for h in range(HL):
                nc.sync.dma_start(out=wo_s[:, h, :], in_=wo[h, :, :])

            # ---- Q projection: qT[:, h, :] = Wq_h.T @ xT ----
            for h in range(HL):
                for ih in range(NQ // 512):
                    acc = ps.tile([128, 512], F32, tag="pp", bufs=2,
                                  name=f"accq_{h}_{ih}")
                    for k in range(KT):
                        nc.tensor.matmul(
                            acc,
                            wq_s[:, k, h * DH:(h + 1) * DH],
                            xt[:, k, ih * 512:(ih + 1) * 512],
                            start=(k == 0), stop=(k == KT - 1))
                    nc.vector.tensor_copy(qT[:, h, ih * 512:(ih + 1) * 512], acc)

            # ---- K projection: kT[:, h, :] = Wk_h.T @ condT ----
            for h in range(HL):
                for jh in range(NK // 512):
                    acc = ps.tile([128, 512], F32, tag="pp", bufs=2,
                                  name=f"acck_{h}_{jh}")
                    for k in range(KT):
                        nc.tensor.matmul(
                            acc,
                            wk_s[:, k, h * DH:(h + 1) * DH],
                            ct[:, k, jh * 512:(jh + 1) * 512],
                            start=(k == 0), stop=(k == KT - 1))
                    nc.vector.tensor_copy(kT[:, h, jh * 512:(jh + 1) * 512], acc)

            # ---- V projection: v[:, jt, :] = condT_jt.T @ Wv (4 heads wide) ----
            for jt in range(JT):
                acc = ps.tile([128, MW], F32, tag="pp", bufs=2,
                              name=f"accv_{jt}")
                for k in range(KT):
                    nc.tensor.matmul(
                        acc,
                        ct[:, k, jt * 128:(jt + 1) * 128],
                        wv_s[:, k, :],
                        start=(k == 0), stop=(k == KT - 1))
                nc.vector.tensor_copy(v[:, jt, :], acc)

            # output accumulator recycles xt's SBUF (xt is dead after Q proj)
            out_acc = big.tile([128, IT, D], F16, tag="xt")

            # ---- attention per head (two 1024-query halves each) ----
            for h in range(HL):
                for half in range(2):
                    i0 = half * 1024
                    avs = [ps.tile([128, 512], F32, tag="av", bufs=2,
                                   name=f"av_{h}_{half}_{i}") for i in range(2)]
                    den_s = denp.tile([128, 1024], F16, tag="den_s",
                                      name=f"den_s_{h}_{half}")
                    for jt in range(JT):
                        sc = ps.tile([128, 1024], F32, tag="sc", bufs=2,
                                     name=f"sc_{h}_{half}_{jt}")
                        for ih in range(2):
                            nc.tensor.matmul(
                                sc[:, ih * 512:(ih + 1) * 512],
                                kT[:, h, jt * 128:(jt + 1) * 128],
                                qT[:, h, i0 + ih * 512:i0 + (ih + 1) * 512],
                                start=True, stop=True)
                        esc = expp.tile([128, 1024], F16, tag="esc",
                                        name=f"esc_{h}_{half}_{jt}")
                        nc.scalar.activation(esc, sc, EXP)
                        for ih in range(2):
                            nc.tensor.matmul(
                                avs[ih],
                                v[:, jt, h * DH:(h + 1) * DH],
                                esc[:, ih * 512:(ih + 1) * 512],
                                start=(jt == 0), stop=(jt == JT - 1))
                        with tc.high_priority():
                            if jt == 0:
                                nc.vector.tensor_copy(den_s, esc)
                            else:
                                nc.vector.tensor_add(den_s, den_s, esc)
                    den_bc = ps.tile([128, 1024], F32, tag="sc", bufs=2,
                                     name=f"den_bc_{h}_{half}")
                    den_rec = denp.tile([128, 1024], F32, tag="den_rec",
                                        name=f"den_rec_{h}_{half}")
                    with tc.high_priority():
                        for ih in range(2):
                            nc.tensor.matmul(
                                den_bc[:, ih * 512:(ih + 1) * 512],
                                ones,
                                den_s[:, ih * 512:(ih + 1) * 512],
                                start=True, stop=True)
                        nc.vector.reciprocal_approx_fast(out=den_rec, in_=den_bc)
                        for ih in range(2):
                            nc.vector.tensor_mul(
                                attT[:, h, i0 + ih * 512:i0 + (ih + 1) * 512],
                                avs[ih],
                                den_rec[:, ih * 512:(ih + 1) * 512])

                # ---- partial O projection contribution of head h ----
                # out_acc[:, it, :] (+)= attT_h[:, it-tile].T @ Wo_h
                for it in range(IT):
                    for nh in range(2):
                        fo = ps.tile([128, 512], F32, tag="pp", bufs=2,
                                     name=f"fo_{h}_{it}_{nh}")
                        nc.tensor.matmul(
                            fo,
                            attT[:, h, it * 128:(it + 1) * 128],
                            wo_s[:, h, nh * 512:(nh + 1) * 512],
                            start=True, stop=True)
                        dst = out_acc[:, it, nh * 512:(nh + 1) * 512]
                        if h == 0:
                            nc.vector.tensor_copy(dst, fo)
                        else:
                            nc.vector.tensor_add(dst, dst, fo)

            for it in range(IT):
                nc.sync.dma_start(out=out[it * 128:(it + 1) * 128, :],
                                  in_=out_acc[:, it, :])
    nc.finalize()
    return nc


_NC_CACHE = None


def _get_nc():
    global _NC_CACHE
    if _NC_CACHE is None:
        _NC_CACHE = build_nc()
    return _NC_CACHE


def make_in_maps(x, cond, Wq, Wk, Wv, Wo, bo):
    wq16 = (np.asarray(Wq, np.float32) * SCALE).astype(np.float16)
    wk16 = np.asarray(Wk, np.float32).astype(np.float16)
    wv16 = np.asarray(Wv, np.float32).astype(np.float16)
    wo16 = np.asarray(Wo, np.float32).astype(np.float16)
    x16 = np.asarray(x, np.float32).astype(np.float16)
    c16 = np.asarray(cond, np.float32).astype(np.float16)
    in_maps = []
    for c in range(NCORES):
        b, hh = c // 2, c % 2
        cols = slice(hh * MW, (hh + 1) * MW)
        in_maps.append({
            "xT": np.ascontiguousarray(x16[b].T).reshape(KT, 128, NQ),
            "condT": np.ascontiguousarray(c16[b].T).reshape(KT, 128, NK),
            "wq": np.ascontiguousarray(wq16[:, cols]).reshape(KT, 128, MW),
            "wk": np.ascontiguousarray(wk16[:, cols]).reshape(KT, 128, MW),
            "wv": np.ascontiguousarray(wv16[:, cols]).reshape(KT, 128, MW),
            "wo": np.ascontiguousarray(wo16[cols, :]).reshape(HL, 128, D),
        })
    return in_maps


def kernel(x, cond, Wq, Wk, Wv, Wo, bo, _trace=False, _trace_kwargs=None):
    nc = _get_nc()
    in_maps = make_in_maps(x, cond, Wq, Wk, Wv, Wo, bo)
    kw = {}
    if _trace:
        kw = {"trace": True, "trace_kwargs": _trace_kwargs or {}}
    res = run_bass_kernel_spmd(nc, in_maps, list(range(NCORES)), **kw)
    bo32 = np.asarray(bo, np.float32)
    out = np.empty((B, NQ, D), dtype=np.float32)
    for b in range(B):
        out[b] = (res.results[2 * b]["out"].astype(np.float32)
                  + res.results[2 * b + 1]["out"].astype(np.float32) + bo32)
    if _trace:
        return out, res
    return out


if __name__ == "__main__":
    rng = np.random.default_rng(0)
    s = 0.02
    x = rng.standard_normal((B, NQ, D), dtype=np.float32)
    cond = rng.standard_normal((B, NK, D), dtype=np.float32)
    Wq = (rng.standard_normal((D, D), dtype=np.float32) * s)
    Wk = (rng.standard_normal((D, D), dtype=np.float32) * s)
    Wv = (rng.standard_normal((D, D), dtype=np.float32) * s)
    Wo = (rng.standard_normal((D, D), dtype=np.float32) * s)
    bo = (rng.standard_normal((D,), dtype=np.float32) * s)

    def ref_np(x, cond):
        q = (x @ Wq).reshape(B, NQ, H, DH).transpose(0, 2, 1, 3)
        k = (cond @ Wk).reshape(B, NK, H, DH).transpose(0, 2, 1, 3)
        v = (cond @ Wv).reshape(B, NK, H, DH).transpose(0, 2, 1, 3)
        sim = np.einsum('bhid,bhjd->bhij', q, k) * SCALE
        sim = sim - sim.max(axis=-1, keepdims=True)
        a = np.exp(sim)
        a = a / a.sum(axis=-1, keepdims=True)
        o = np.einsum('bhij,bhjd->bhid', a, v)
        o = o.transpose(0, 2, 1, 3).reshape(B, NQ, D)
        return o @ Wo + bo

    import time
    t0 = time.time()
    got = kernel(x=x, cond=cond, Wq=Wq, Wk=Wk, Wv=Wv, Wo=Wo, bo=bo)
    print(f"kernel run {time.time()-t0:.1f}s")
    exp = ref_np(x.astype(np.float64), cond.astype(np.float64))
    err = np.abs(got - exp)
    rel = np.linalg.norm(got - exp) / np.linalg.norm(exp)
    print(f"rel_l2={rel:.3e} absmax_rel={err.max()/np.abs(exp).max():.3e}")
